# revision 10
# baseline (speedup 1.0000x reference)
"""Multi-head attention on 8 TRN2 NeuronCores (data/head-parallel).

Problem: B=4 H=16 S=2048 D=64 fp32 attention, out = softmax(Q K^T / sqrt(D)) V.
B*H = 64 (batch, head) pairs are sharded 8 per core; each core runs the same
NEFF over its own 8 heads, no collectives.

Design (v3):
  - QK^T uses 2-way PE row tiling (64x128 mode, array tiles T0/T8). The
    d=64 contraction fills only half the 128-row PE array, so k-tile 2j's
    K^T sits in rows 0:64 and k-tile 2j+1's in rows 64:128; Q^T is
    host-duplicated into both SBUF partition halves and the two score
    matmuls stream concurrently => ~2x QK throughput.
  - PV is ALSO row-tiled 64x128 (v3 change): V' tile t is split into k-rows
    0:64 (T0) and 64:128 (T8); the two halves stream et[0:64]/et[64:128]
    concurrently, accumulating into two separate PSUM banks (row-tiled
    matmuls must not co-write one bank). This keeps the whole kernel in one
    PE tiling mode - the baseline's 64x128 <-> 128x128 mode switches cost
    ~150-250ns of pipeline drain each, ~19us/core total.
  - The two PV partial banks are merged by a single DVE tensor_tensor add
    (psum+psum -> sbuf) which replaces the baseline's copy at equal cost.
  - exp runs on two engines: ACT computes exact exp for 9 of 16 score
    tiles per super-block; the other 7 are a one-instruction Schraudolph
    exponential on the Vector engine: i16 = round(score * 128*log2(e)/
    sqrt(D) + 16248.5) IS the bf16 bit pattern of exp(score/sqrt(D)) to
    ~1.8% rms; the PV matmul reads it through a bitcast view. Softmax's
    scale invariance cancels the common-mode part of that error.
  - PE warmup: ~8 dummy 512-col matmuls on a memset scratch tile fill the
    DMA dead-time at kernel start so the PE_HAM clock gate reaches 8/8
    (2.4 GHz) before the first real matmul (else the first ~3.4us of real
    matmuls run at 1.2 GHz).
  - V gets a ones column appended, so PV's PSUM accumulators hold the
    unnormalized output transpose with softmax denominators in row 64. The
    division and [d, q] -> [q, d] transpose happen on the host.
  - PSUM budget (8 banks): 2 x score tiles [128,1024]f32 (2 banks each) +
    4 x PV accumulators [128,512]f32 = 8.
  - kt/vp are host-packed so every DMA is contiguous per partition.
"""

import math
import os
from contextlib import ExitStack

import ml_dtypes
import numpy as np

import concourse.bass as bass
import concourse.bacc as bacc
import concourse.tile as tile
import concourse.mybir as mybir
from concourse.bass_utils import run_bass_kernel_spmd

B, H, S, D = 4, 16, 2048, 64
N_CORES = 8
HPC = B * H // N_CORES     # heads per core
NPAIR = 8                  # k-tile pairs (16 k-tiles of 128)
QB = 512                   # q sub-block (one PSUM bank of scores per k-tile)
SB = 1024                  # q super-block (one stationary load per k-tile)
NSB = S // SB
DT = mybir.dt

# Schraudolph-in-bf16-bit-space constants: exp(score/8) ~= bits of
# int16(score * EXP_A + EXP_B). EXP_B calibrated for round-to-nearest.
EXP_A = 128.0 * 1.4426950408889634 / 8.0
EXP_B = 16248.5
# (pair, sub-block) slots whose exp runs on DVE instead of ACT (7 of 16).
DVE_SLOTS = ((1, 1), (2, 0), (3, 1), (4, 0), (5, 1), (6, 0), (7, 1))

# --- tuning knobs (env-overridable for A/B) ---
PV_ROW_TILED = os.environ.get("PV_ROW_TILED", "1") == "1"
WARMUP_MMS = int(os.environ.get("WARMUP_MMS", "8"))
ST_BUFS = int(os.environ.get("ST_BUFS", "2" if PV_ROW_TILED else "3"))
OT_BUFS = int(os.environ.get("OT_BUFS", "2"))
# interleave cadence: one PV chunk (8 MMs) per PV_EVERY QK pairs. Row-tiled
# PV has 64 MMs per super-block (vs 32 classic) so it interleaves per-pair.
PV_EVERY = 1 if PV_ROW_TILED else 2

_BUILT = {}


class _Bacc(bacc.Bacc):
    """Bacc with the move-matmul-waits-to-ldweights pass disabled: keeping
    waits on the matmul (not its LDWEIGHTS) lets the PE queue pull weight
    loads ahead of in-flight matmuls, hiding most of the LDW cost, and
    avoids walrus folding LDW waits into the matmul's sync-wait budget."""

    def move_matmul_waits_to_ldweights(self):
        pass


def _head(nc, pools, scale, qt_d, kt_d, vp_d, h):
    """Emit head h's loads + per-super-block QK/exp. Yields None at each
    2-QK-group boundary (interleave point for the previous super-block's
    PV) and (vp, ets, q0) when a super-block's score tiles are emitted."""
    (stage, epool, spool, ps_st, ps_ot) = pools
    # qt rows 0:64 / 64:128 both hold Q^T. kt packs k-tile pairs: rows
    # 0:64 = K^T of tile 2j, rows 64:128 = K^T of tile 2j+1. vp is
    # partition-major: vp[p, t, e] = V'[t*128 + p, e].
    qt = stage.tile([128, S], DT.bfloat16, tag="qt")
    kt = stage.tile([128, NPAIR * 128], DT.bfloat16, tag="kt")
    vp = stage.tile([128, 2 * NPAIR, 128], DT.bfloat16, tag="vp")
    if h == 0:
        # first QK group needs only kt pair 0 (32KB) + qt[0:512]; land
        # those first (on two queues in parallel) so the PE starts early.
        nc.sync.dma_start(out=kt[:, 0:128], in_=kt_d[h][:, 0:128])
        nc.gpsimd.dma_start(out=qt[:, 0:QB], in_=qt_d[h][:, 0:QB])
        nc.sync.dma_start(out=kt[:, 128:], in_=kt_d[h][:, 128:])
        nc.sync.dma_start(out=qt[:, QB:SB], in_=qt_d[h][:, QB:SB])
        nc.gpsimd.dma_start(out=qt[:, SB:], in_=qt_d[h][:, SB:])
    else:
        nc.gpsimd.dma_start(out=kt, in_=kt_d[h])
        for j in range(2):
            half = slice(j * (S // 2), (j + 1) * (S // 2))
            nc.gpsimd.dma_start(out=qt[:, half], in_=qt_d[h][:, half])
    nc.gpsimd.dma_start(out=vp,
                        in_=vp_d[h].rearrange("p (t e) -> p t e", e=128))

    for c in range(NSB):
        q0 = c * SB
        ets = []
        for j in range(NPAIR):
            if j % PV_EVERY == 0:
                yield None
            st0 = ps_st.tile([128, 2 * QB], DT.float32, tag="st")
            st1 = ps_st.tile([128, 2 * QB], DT.float32, tag="st")
            # T0: k-tile 2j via rows 0:64; T8: k-tile 2j+1 via rows
            # 64:128, concurrent. Sub-blocks fill each st tile's 2 banks.
            for s, st in enumerate((st0, st1)):
                nc.tensor.matmul(
                    st[:, 0:QB],
                    lhsT=kt[0:64, j * 128 : (j + 1) * 128],
                    rhs=qt[0:64, q0 + s * QB : q0 + (s + 1) * QB],
                    start=True, stop=True,
                )
            for s, st in enumerate((st0, st1)):
                nc.tensor.matmul(
                    st[:, QB : 2 * QB],
                    lhsT=kt[64:128, j * 128 : (j + 1) * 128],
                    rhs=qt[64:128, q0 + s * QB : q0 + (s + 1) * QB],
                    start=True, stop=True,
                )
            for s, st in enumerate((st0, st1)):
                et = epool.tile([128, 2 * QB], DT.bfloat16, tag=f"et{j}_{s}")
                if (j, s) in DVE_SLOTS:
                    nc.vector.tensor_scalar(
                        et.bitcast(DT.int16), st, EXP_A, EXP_B,
                        mybir.AluOpType.mult, mybir.AluOpType.add,
                    )
                else:
                    nc.scalar.activation(
                        out=et, in_=st,
                        func=mybir.ActivationFunctionType.Exp, scale=scale,
                    )
                ets.append(et)
        yield (vp, list(ets), q0)


def _pv_gen_row(nc, pools, vp, ets, o_ap, q0):
    """Row-tiled PV for one super-block (64x128 mode, same as QK => no PE
    tiling-mode switches). V' tile t is split into k-rows 0:64 (T0) and
    64:128 (T8); both halves stream their et half concurrently. T0/T8
    accumulate into separate 2-bank PSUM tiles (cols 0:512 = sub-block 0,
    512:1024 = sub-block 1). Merged at the end by an ACT Copy (lo -> SBUF)
    plus a DVE add (SBUF + hi -> SBUF): a DVE op may read at most one PSUM
    operand, so the two partial banks cannot be added in one instruction.
    Yields every 8 matmuls."""
    (stage, epool, spool, ps_st, ps_ot) = pools
    ot_lo = ps_ot.tile([128, 2 * QB], DT.float32, tag="ot")
    ot_hi = ps_ot.tile([128, 2 * QB], DT.float32, tag="ot")
    k = 0
    for j in range(NPAIR):
        for half in range(2):
            t = 2 * j + half
            for s in range(2):
                et = ets[2 * j + s]
                for g, (rows, ot) in enumerate(
                        ((slice(0, 64), ot_lo), (slice(64, 128), ot_hi))):
                    nc.tensor.matmul(
                        ot[:, s * QB : (s + 1) * QB],
                        lhsT=vp[rows, t, :],
                        rhs=et[rows, half * QB : (half + 1) * QB],
                        start=(t == 0), stop=(t == 2 * NPAIR - 1),
                    )
                    k += 1
                    if k % 8 == 0:
                        yield
    # unnormalized out^T: rows 0:64 numerator, row 64 denominator.
    otmp = spool.tile([D + 1, 2 * QB], DT.float32, tag="otmp")
    nc.scalar.activation(out=otmp, in_=ot_lo[0 : D + 1, :],
                         func=mybir.ActivationFunctionType.Copy)
    ots = spool.tile([D + 1, 2 * QB], DT.float32, tag="ots")
    nc.vector.tensor_tensor(out=ots, in0=otmp, in1=ot_hi[0 : D + 1, :],
                            op=mybir.AluOpType.add)
    nc.sync.dma_start(out=o_ap[:, q0 : q0 + SB], in_=ots)


def _pv_gen_classic(nc, pools, vp, ets, o_ap, q0):
    """Baseline PV (128x128 mode), yielding every 8 matmuls."""
    (stage, epool, spool, ps_st, ps_ot) = pools
    k = 0
    for s in range(2):
        ot = ps_ot.tile([128, QB], DT.float32, tag="ot")
        for j in range(NPAIR):
            for half in range(2):
                t = 2 * j + half
                nc.tensor.matmul(
                    ot,
                    lhsT=vp[:, t, :],
                    rhs=ets[2 * j + s][:, half * QB : (half + 1) * QB],
                    start=(t == 0), stop=(t == 2 * NPAIR - 1),
                )
                k += 1
                if k % 8 == 0:
                    yield
        ots = spool.tile([D + 1, QB], DT.float32, tag="ots")
        nc.vector.tensor_copy(out=ots, in_=ot[0 : D + 1, :])
        nc.sync.dma_start(out=o_ap[:, q0 + s * QB : q0 + (s + 1) * QB],
                          in_=ots)


def build_graph(scale: float, heads: int = HPC):
    nc = _Bacc("TRN2", target_bir_lowering=False, debug=False,
               num_devices=N_CORES)
    qt_d = nc.dram_tensor("QT", [heads, 128, S], DT.bfloat16,
                          kind="ExternalInput").ap()
    kt_d = nc.dram_tensor("KT", [heads, 128, NPAIR * 128], DT.bfloat16,
                          kind="ExternalInput").ap()
    vp_d = nc.dram_tensor("VP", [heads, 128, 2 * NPAIR * 128], DT.bfloat16,
                          kind="ExternalInput").ap()
    o_d = nc.dram_tensor("out", [heads, D + 1, S], DT.float32,
                         kind="ExternalOutput").ap()

    pv_gen = _pv_gen_row if PV_ROW_TILED else _pv_gen_classic

    with tile.TileContext(nc) as tc, ExitStack() as ctx:
        stage = ctx.enter_context(tc.tile_pool(name="stage", bufs=2))
        epool = ctx.enter_context(tc.tile_pool(name="epool", bufs=2))
        spool = ctx.enter_context(tc.tile_pool(name="spool", bufs=4))
        ps_st = ctx.enter_context(tc.tile_pool(name="ps_st", bufs=ST_BUFS,
                                               space="PSUM"))
        ps_ot = ctx.enter_context(tc.tile_pool(name="ps_ot", bufs=OT_BUFS,
                                               space="PSUM"))

        pools = (stage, epool, spool, ps_st, ps_ot)

        if WARMUP_MMS:
            # Warm the PE_HAM clock gate during the initial DMA wait: dummy
            # matmuls on a memset scratch tile into a recycled psum tile.
            wsrc = stage.tile([128, 128 + QB], DT.bfloat16, tag="warm")
            nc.vector.memset(wsrc, 0.0)
            wps = ps_st.tile([128, 2 * QB], DT.float32, tag="st")
            for _ in range(WARMUP_MMS):
                nc.tensor.matmul(wps[:, 0:QB], lhsT=wsrc[:, 0:128],
                                 rhs=wsrc[:, 128:], start=True, stop=True)

        prev = None
        for h in range(HPC):
            for item in _head(nc, pools, scale, qt_d, kt_d, vp_d, h):
                if item is None:
                    if prev is not None:
                        next(prev, None)   # 8 PV MMs of super-block g-1
                    continue
                if prev is not None:
                    for _ in prev:         # PV tail + merges + stores
                        pass
                prev = pv_gen(nc, pools, item[0], item[1], o_d[h], item[2])
        if prev is not None:
            for _ in prev:                 # drain the final super-block
                pass
    nc.compile()
    return nc


def _get_nc(scale: float):
    key = (round(float(scale), 9), PV_ROW_TILED, WARMUP_MMS, ST_BUFS, OT_BUFS)
    if key not in _BUILT:
        _BUILT[key] = build_graph(float(scale))
    return _BUILT[key]


def shard_inputs(Q, K, V):
    """Host-side prep: shard heads across cores; build qt (Q^T duplicated
    into both 64-row halves), kt (k-tile pairs packed for row tiling), vp
    (V plus a ones column, partition-major so every DMA is contiguous)."""
    bf16 = ml_dtypes.bfloat16
    qs = np.asarray(Q, dtype=np.float32).reshape(B * H, S, D)
    ks = np.asarray(K, dtype=np.float32).reshape(B * H, S, D)
    vs = np.asarray(V, dtype=np.float32).reshape(B * H, S, D)
    qtT = qs.transpose(0, 2, 1).astype(bf16)           # [BH, D, S]
    qt = np.concatenate([qtT, qtT], axis=1)            # [BH, 128, S]
    ktT = ks.transpose(0, 2, 1).astype(bf16)           # [BH, D, S]
    ktv = ktT.reshape(B * H, D, NPAIR, 2, 128)
    kt = np.empty((B * H, 128, NPAIR, 128), dtype=bf16)
    kt[:, :D] = ktv[:, :, :, 0, :]                     # rows 0:64 <- 2j
    kt[:, D:] = ktv[:, :, :, 1, :]                     # rows 64:128 <- 2j+1
    kt = kt.reshape(B * H, 128, NPAIR * 128)
    vpb = np.zeros((B * H, S, 128), dtype=bf16)
    vpb[:, :, :D] = vs.astype(bf16)
    vpb[:, :, D] = np.float32(1.0)
    # vp[bh, p, t*128 + e] = V'[bh, t*128 + p, e]
    vp = (vpb.reshape(B * H, 2 * NPAIR, 128, 128)
          .transpose(0, 2, 1, 3).reshape(B * H, 128, 2 * NPAIR * 128))
    in_maps = []
    for c in range(N_CORES):
        sl = slice(c * HPC, (c + 1) * HPC)
        in_maps.append({
            "QT": np.ascontiguousarray(qt[sl]),
            "KT": np.ascontiguousarray(kt[sl]),
            "VP": np.ascontiguousarray(vp[sl]),
        })
    return in_maps


def kernel(Q, K, V, d_k, **run_kwargs):
    scale = 1.0 / math.sqrt(float(d_k))
    nc = _get_nc(scale)
    in_maps = shard_inputs(Q, K, V)
    res = run_bass_kernel_spmd(nc, in_maps, core_ids=list(range(N_CORES)),
                               **run_kwargs)
    # device output is [heads, 65, S]: rows 0:64 = (sum_k p*V)^T, row 64 =
    # softmax denominator. Normalize + transpose on the host.
    outs = []
    for r in res.results:
        o = r["out"]
        outs.append((o[:, :D, :] / o[:, D : D + 1, :]).transpose(0, 2, 1))
    out = np.concatenate(outs, axis=0).reshape(B, H, S, D)
    out = np.ascontiguousarray(out, dtype=np.float32)
    kernel.last_results = res
    return out


# revision 12
# speedup vs baseline: 1.0693x; 1.0693x over previous
"""Multi-head attention on 8 TRN2 NeuronCores (data/head-parallel).

Problem: B=4 H=16 S=2048 D=64 fp32 attention, out = softmax(Q K^T / sqrt(D)) V.
B*H = 64 (batch, head) pairs are sharded 8 per core; each core runs the same
NEFF over its own 8 heads, no collectives.

Design (v3):
  - QK^T uses 2-way PE row tiling (64x128 mode, array tiles T0/T8). The
    d=64 contraction fills only half the 128-row PE array, so k-tile 2j's
    K^T sits in rows 0:64 and k-tile 2j+1's in rows 64:128; Q^T is
    host-duplicated into both SBUF partition halves and the two score
    matmuls stream concurrently => ~2x QK throughput.
  - PV is ALSO row-tiled 64x128 (v3 change): V' tile t is split into k-rows
    0:64 (T0) and 64:128 (T8); the two halves stream et[0:64]/et[64:128]
    concurrently, accumulating into two separate PSUM banks (row-tiled
    matmuls must not co-write one bank). This keeps the whole kernel in one
    PE tiling mode - the baseline's 64x128 <-> 128x128 mode switches cost
    ~150-250ns of pipeline drain each, ~19us/core total.
  - The two PV partial banks are merged by a single DVE tensor_tensor add
    (psum+psum -> sbuf) which replaces the baseline's copy at equal cost.
  - exp runs on two engines: ACT computes exact exp for 9 of 16 score
    tiles per super-block; the other 7 are a one-instruction Schraudolph
    exponential on the Vector engine: i16 = round(score * 128*log2(e)/
    sqrt(D) + 16248.5) IS the bf16 bit pattern of exp(score/sqrt(D)) to
    ~1.8% rms; the PV matmul reads it through a bitcast view. Softmax's
    scale invariance cancels the common-mode part of that error.
  - PE warmup: ~8 dummy 512-col matmuls on a memset scratch tile fill the
    DMA dead-time at kernel start so the PE_HAM clock gate reaches 8/8
    (2.4 GHz) before the first real matmul (else the first ~3.4us of real
    matmuls run at 1.2 GHz).
  - V gets a ones column appended, so PV's PSUM accumulators hold the
    unnormalized output transpose with softmax denominators in row 64. The
    division and [d, q] -> [q, d] transpose happen on the host.
  - PSUM budget (8 banks): 2 x score tiles [128,1024]f32 (2 banks each) +
    4 x PV accumulators [128,512]f32 = 8.
  - kt/vp are host-packed so every DMA is contiguous per partition.
"""

import math
import os
from contextlib import ExitStack

import ml_dtypes
import numpy as np

import concourse.bass as bass
import concourse.bacc as bacc
import concourse.tile as tile
import concourse.mybir as mybir
from concourse.bass_utils import run_bass_kernel_spmd

B, H, S, D = 4, 16, 2048, 64
N_CORES = 8
HPC = B * H // N_CORES     # heads per core
NPAIR = 8                  # k-tile pairs (16 k-tiles of 128)
QB = 512                   # q sub-block (one PSUM bank of scores per k-tile)
SB = 1024                  # q super-block (one stationary load per k-tile)
NSB = S // SB
DT = mybir.dt

# Schraudolph-in-bf16-bit-space constants: exp(score/8) ~= bits of
# int16(score * EXP_A + EXP_B). EXP_B calibrated for round-to-nearest.
EXP_A = 128.0 * 1.4426950408889634 / 8.0
EXP_B = 16248.5
# (pair, sub-block) slots whose exp runs on DVE instead of ACT (7 of 16).
DVE_SLOTS = ((1, 1), (2, 0), (3, 1), (4, 0), (5, 1), (6, 0), (7, 1))

# --- tuning knobs (env-overridable for A/B) ---
PV_ROW_TILED = os.environ.get("PV_ROW_TILED", "0") == "1"
WARMUP_MMS = int(os.environ.get("WARMUP_MMS", "8"))
ST_BUFS = int(os.environ.get("ST_BUFS", "2" if PV_ROW_TILED else "3"))
OT_BUFS = int(os.environ.get("OT_BUFS", "2"))
# interleave cadence: one PV chunk (8 MMs) per PV_EVERY QK pairs. Row-tiled
# PV has 64 MMs per super-block (vs 32 classic) so it interleaves per-pair.
PV_EVERY = 1 if PV_ROW_TILED else 2

_BUILT = {}


class _Bacc(bacc.Bacc):
    """Bacc with the move-matmul-waits-to-ldweights pass disabled: keeping
    waits on the matmul (not its LDWEIGHTS) lets the PE queue pull weight
    loads ahead of in-flight matmuls, hiding most of the LDW cost, and
    avoids walrus folding LDW waits into the matmul's sync-wait budget."""

    def move_matmul_waits_to_ldweights(self):
        pass


def _head(nc, pools, scale, qt_d, kt_d, vp_d, h):
    """Emit head h's loads + per-super-block QK/exp. Yields None at each
    2-QK-group boundary (interleave point for the previous super-block's
    PV) and (vp, ets, q0) when a super-block's score tiles are emitted."""
    (stage, epool, spool, ps_st, ps_ot) = pools
    # qt rows 0:64 / 64:128 both hold Q^T. kt packs k-tile pairs: rows
    # 0:64 = K^T of tile 2j, rows 64:128 = K^T of tile 2j+1. vp is
    # partition-major: vp[p, t, e] = V'[t*128 + p, e].
    qt = stage.tile([128, S], DT.bfloat16, tag="qt")
    kt = stage.tile([128, NPAIR * 128], DT.bfloat16, tag="kt")
    vp = stage.tile([128, 2 * NPAIR, 128], DT.bfloat16, tag="vp")
    if h == 0:
        # first QK group needs only kt pair 0 (32KB) + qt[0:512]; land
        # those first (on two queues in parallel) so the PE starts early.
        nc.sync.dma_start(out=kt[:, 0:128], in_=kt_d[h][:, 0:128])
        nc.gpsimd.dma_start(out=qt[:, 0:QB], in_=qt_d[h][:, 0:QB])
        nc.sync.dma_start(out=kt[:, 128:], in_=kt_d[h][:, 128:])
        nc.sync.dma_start(out=qt[:, QB:SB], in_=qt_d[h][:, QB:SB])
        nc.gpsimd.dma_start(out=qt[:, SB:], in_=qt_d[h][:, SB:])
    else:
        nc.gpsimd.dma_start(out=kt, in_=kt_d[h])
        for j in range(2):
            half = slice(j * (S // 2), (j + 1) * (S // 2))
            nc.gpsimd.dma_start(out=qt[:, half], in_=qt_d[h][:, half])
    nc.gpsimd.dma_start(out=vp,
                        in_=vp_d[h].rearrange("p (t e) -> p t e", e=128))

    for c in range(NSB):
        q0 = c * SB
        ets = []
        for j in range(NPAIR):
            if j % PV_EVERY == 0:
                yield None
            st0 = ps_st.tile([128, 2 * QB], DT.float32, tag="st")
            st1 = ps_st.tile([128, 2 * QB], DT.float32, tag="st")
            # T0: k-tile 2j via rows 0:64; T8: k-tile 2j+1 via rows
            # 64:128, concurrent. Sub-blocks fill each st tile's 2 banks.
            # Row groups alternate (T0,T8,T0,T8) so every LDWEIGHTS (bass
            # re-emits one per matmul, even for repeated weights) overlaps
            # the other group's in-flight matmul instead of serializing.
            for s, st in enumerate((st0, st1)):
                nc.tensor.matmul(
                    st[:, 0:QB],
                    lhsT=kt[0:64, j * 128 : (j + 1) * 128],
                    rhs=qt[0:64, q0 + s * QB : q0 + (s + 1) * QB],
                    start=True, stop=True,
                )
                nc.tensor.matmul(
                    st[:, QB : 2 * QB],
                    lhsT=kt[64:128, j * 128 : (j + 1) * 128],
                    rhs=qt[64:128, q0 + s * QB : q0 + (s + 1) * QB],
                    start=True, stop=True,
                )
            for s, st in enumerate((st0, st1)):
                et = epool.tile([128, 2 * QB], DT.bfloat16, tag=f"et{j}_{s}")
                if (j, s) in DVE_SLOTS:
                    nc.vector.tensor_scalar(
                        et.bitcast(DT.int16), st, EXP_A, EXP_B,
                        mybir.AluOpType.mult, mybir.AluOpType.add,
                    )
                else:
                    nc.scalar.activation(
                        out=et, in_=st,
                        func=mybir.ActivationFunctionType.Exp, scale=scale,
                    )
                ets.append(et)
        yield (vp, list(ets), q0)


def _pv_gen_row(nc, pools, vp, ets, o_ap, q0):
    """Row-tiled PV for one super-block (64x128 mode, same as QK => no PE
    tiling-mode switches). V' tile t is split into k-rows 0:64 (T0) and
    64:128 (T8); both halves stream their et half concurrently. T0/T8
    accumulate into separate 2-bank PSUM tiles (cols 0:512 = sub-block 0,
    512:1024 = sub-block 1). Merged at the end by an ACT Copy (lo -> SBUF)
    plus a DVE add (SBUF + hi -> SBUF): a DVE op may read at most one PSUM
    operand, so the two partial banks cannot be added in one instruction.
    Yields every 8 matmuls."""
    (stage, epool, spool, ps_st, ps_ot) = pools
    ot_lo = ps_ot.tile([128, 2 * QB], DT.float32, tag="ot")
    ot_hi = ps_ot.tile([128, 2 * QB], DT.float32, tag="ot")
    k = 0
    for j in range(NPAIR):
        for half in range(2):
            t = 2 * j + half
            for s in range(2):
                et = ets[2 * j + s]
                for g, (rows, ot) in enumerate(
                        ((slice(0, 64), ot_lo), (slice(64, 128), ot_hi))):
                    nc.tensor.matmul(
                        ot[:, s * QB : (s + 1) * QB],
                        lhsT=vp[rows, t, :],
                        rhs=et[rows, half * QB : (half + 1) * QB],
                        start=(t == 0), stop=(t == 2 * NPAIR - 1),
                    )
                    k += 1
                    if k % 8 == 0:
                        yield
    # unnormalized out^T: rows 0:64 numerator, row 64 denominator.
    otmp = spool.tile([D + 1, 2 * QB], DT.float32, tag="otmp")
    nc.scalar.activation(out=otmp, in_=ot_lo[0 : D + 1, :],
                         func=mybir.ActivationFunctionType.Copy)
    ots = spool.tile([D + 1, 2 * QB], DT.float32, tag="ots")
    nc.vector.tensor_tensor(out=ots, in0=otmp, in1=ot_hi[0 : D + 1, :],
                            op=mybir.AluOpType.add)
    nc.sync.dma_start(out=o_ap[:, q0 : q0 + SB], in_=ots)


def _pv_gen_classic(nc, pools, vp, ets, o_ap, q0):
    """Baseline PV (128x128 mode), yielding every 8 matmuls."""
    (stage, epool, spool, ps_st, ps_ot) = pools
    k = 0
    for s in range(2):
        ot = ps_ot.tile([128, QB], DT.float32, tag="ot")
        for j in range(NPAIR):
            for half in range(2):
                t = 2 * j + half
                nc.tensor.matmul(
                    ot,
                    lhsT=vp[:, t, :],
                    rhs=ets[2 * j + s][:, half * QB : (half + 1) * QB],
                    start=(t == 0), stop=(t == 2 * NPAIR - 1),
                )
                k += 1
                if k % 8 == 0:
                    yield
        ots = spool.tile([D + 1, QB], DT.float32, tag="ots")
        nc.vector.tensor_copy(out=ots, in_=ot[0 : D + 1, :])
        nc.sync.dma_start(out=o_ap[:, q0 + s * QB : q0 + (s + 1) * QB],
                          in_=ots)


def build_graph(scale: float, heads: int = HPC):
    nc = _Bacc("TRN2", target_bir_lowering=False, debug=False,
               num_devices=N_CORES)
    qt_d = nc.dram_tensor("QT", [heads, 128, S], DT.bfloat16,
                          kind="ExternalInput").ap()
    kt_d = nc.dram_tensor("KT", [heads, 128, NPAIR * 128], DT.bfloat16,
                          kind="ExternalInput").ap()
    vp_d = nc.dram_tensor("VP", [heads, 128, 2 * NPAIR * 128], DT.bfloat16,
                          kind="ExternalInput").ap()
    o_d = nc.dram_tensor("out", [heads, D + 1, S], DT.float32,
                         kind="ExternalOutput").ap()

    pv_gen = _pv_gen_row if PV_ROW_TILED else _pv_gen_classic

    with tile.TileContext(nc) as tc, ExitStack() as ctx:
        stage = ctx.enter_context(tc.tile_pool(name="stage", bufs=2))
        epool = ctx.enter_context(tc.tile_pool(name="epool", bufs=2))
        spool = ctx.enter_context(tc.tile_pool(name="spool", bufs=4))
        ps_st = ctx.enter_context(tc.tile_pool(name="ps_st", bufs=ST_BUFS,
                                               space="PSUM"))
        ps_ot = ctx.enter_context(tc.tile_pool(name="ps_ot", bufs=OT_BUFS,
                                               space="PSUM"))

        pools = (stage, epool, spool, ps_st, ps_ot)

        if WARMUP_MMS:
            # Warm the PE_HAM clock gate during the initial DMA wait: dummy
            # matmuls on a memset scratch tile into a recycled psum tile.
            wsrc = stage.tile([128, 128 + QB], DT.bfloat16, tag="warm")
            nc.vector.memset(wsrc, 0.0)
            wps = ps_st.tile([128, 2 * QB], DT.float32, tag="st")
            for _ in range(WARMUP_MMS):
                nc.tensor.matmul(wps[:, 0:QB], lhsT=wsrc[:, 0:128],
                                 rhs=wsrc[:, 128:], start=True, stop=True)

        prev = None
        for h in range(HPC):
            for item in _head(nc, pools, scale, qt_d, kt_d, vp_d, h):
                if item is None:
                    if prev is not None:
                        next(prev, None)   # 8 PV MMs of super-block g-1
                    continue
                if prev is not None:
                    for _ in prev:         # PV tail + merges + stores
                        pass
                prev = pv_gen(nc, pools, item[0], item[1], o_d[h], item[2])
        if prev is not None:
            for _ in prev:                 # drain the final super-block
                pass
    nc.compile()
    return nc


def _get_nc(scale: float):
    key = (round(float(scale), 9), PV_ROW_TILED, WARMUP_MMS, ST_BUFS, OT_BUFS)
    if key not in _BUILT:
        _BUILT[key] = build_graph(float(scale))
    return _BUILT[key]


def shard_inputs(Q, K, V):
    """Host-side prep: shard heads across cores; build qt (Q^T duplicated
    into both 64-row halves), kt (k-tile pairs packed for row tiling), vp
    (V plus a ones column, partition-major so every DMA is contiguous)."""
    bf16 = ml_dtypes.bfloat16
    qs = np.asarray(Q, dtype=np.float32).reshape(B * H, S, D)
    ks = np.asarray(K, dtype=np.float32).reshape(B * H, S, D)
    vs = np.asarray(V, dtype=np.float32).reshape(B * H, S, D)
    qtT = qs.transpose(0, 2, 1).astype(bf16)           # [BH, D, S]
    qt = np.concatenate([qtT, qtT], axis=1)            # [BH, 128, S]
    ktT = ks.transpose(0, 2, 1).astype(bf16)           # [BH, D, S]
    ktv = ktT.reshape(B * H, D, NPAIR, 2, 128)
    kt = np.empty((B * H, 128, NPAIR, 128), dtype=bf16)
    kt[:, :D] = ktv[:, :, :, 0, :]                     # rows 0:64 <- 2j
    kt[:, D:] = ktv[:, :, :, 1, :]                     # rows 64:128 <- 2j+1
    kt = kt.reshape(B * H, 128, NPAIR * 128)
    vpb = np.zeros((B * H, S, 128), dtype=bf16)
    vpb[:, :, :D] = vs.astype(bf16)
    vpb[:, :, D] = np.float32(1.0)
    # vp[bh, p, t*128 + e] = V'[bh, t*128 + p, e]
    vp = (vpb.reshape(B * H, 2 * NPAIR, 128, 128)
          .transpose(0, 2, 1, 3).reshape(B * H, 128, 2 * NPAIR * 128))
    in_maps = []
    for c in range(N_CORES):
        sl = slice(c * HPC, (c + 1) * HPC)
        in_maps.append({
            "QT": np.ascontiguousarray(qt[sl]),
            "KT": np.ascontiguousarray(kt[sl]),
            "VP": np.ascontiguousarray(vp[sl]),
        })
    return in_maps


def kernel(Q, K, V, d_k, **run_kwargs):
    scale = 1.0 / math.sqrt(float(d_k))
    nc = _get_nc(scale)
    in_maps = shard_inputs(Q, K, V)
    res = run_bass_kernel_spmd(nc, in_maps, core_ids=list(range(N_CORES)),
                               **run_kwargs)
    # device output is [heads, 65, S]: rows 0:64 = (sum_k p*V)^T, row 64 =
    # softmax denominator. Normalize + transpose on the host.
    outs = []
    for r in res.results:
        o = r["out"]
        outs.append((o[:, :D, :] / o[:, D : D + 1, :]).transpose(0, 2, 1))
    out = np.concatenate(outs, axis=0).reshape(B, H, S, D)
    out = np.ascontiguousarray(out, dtype=np.float32)
    kernel.last_results = res
    return out


# revision 15
# speedup vs baseline: 1.2619x; 1.1801x over previous
"""Multi-head attention on 8 TRN2 NeuronCores (data/head-parallel).

Problem: B=4 H=16 S=2048 D=64 fp32 attention, out = softmax(Q K^T / sqrt(D)) V.
B*H = 64 (batch, head) pairs are sharded 8 per core; each core runs the same
NEFF over its own 8 heads, no collectives.

Design (v3):
  - QK^T uses 2-way PE row tiling (64x128 mode, array tiles T0/T8). The
    d=64 contraction fills only half the 128-row PE array, so k-tile 2j's
    K^T sits in rows 0:64 and k-tile 2j+1's in rows 64:128; Q^T is
    host-duplicated into both SBUF partition halves and the two score
    matmuls stream concurrently => ~2x QK throughput.
  - PV is ALSO row-tiled 64x128 (v3 change): V' tile t is split into k-rows
    0:64 (T0) and 64:128 (T8); the two halves stream et[0:64]/et[64:128]
    concurrently, accumulating into two separate PSUM banks (row-tiled
    matmuls must not co-write one bank). This keeps the whole kernel in one
    PE tiling mode - the baseline's 64x128 <-> 128x128 mode switches cost
    ~150-250ns of pipeline drain each, ~19us/core total.
  - The two PV partial banks are merged by a single DVE tensor_tensor add
    (psum+psum -> sbuf) which replaces the baseline's copy at equal cost.
  - exp runs on two engines: ACT computes exact exp for 9 of 16 score
    tiles per super-block; the other 7 are a one-instruction Schraudolph
    exponential on the Vector engine: i16 = round(score * 128*log2(e)/
    sqrt(D) + 16248.5) IS the bf16 bit pattern of exp(score/sqrt(D)) to
    ~1.8% rms; the PV matmul reads it through a bitcast view. Softmax's
    scale invariance cancels the common-mode part of that error.
  - PE warmup: ~8 dummy 512-col matmuls on a memset scratch tile fill the
    DMA dead-time at kernel start so the PE_HAM clock gate reaches 8/8
    (2.4 GHz) before the first real matmul (else the first ~3.4us of real
    matmuls run at 1.2 GHz).
  - V gets a ones column appended, so PV's PSUM accumulators hold the
    unnormalized output transpose with softmax denominators in row 64. The
    division and [d, q] -> [q, d] transpose happen on the host.
  - PSUM budget (8 banks): 2 x score tiles [128,1024]f32 (2 banks each) +
    4 x PV accumulators [128,512]f32 = 8.
  - kt/vp are host-packed so every DMA is contiguous per partition.
"""

import math
import os
from contextlib import ExitStack

import ml_dtypes
import numpy as np

import concourse.bass as bass
import concourse.bacc as bacc
import concourse.tile as tile
import concourse.mybir as mybir
from concourse.bass_utils import run_bass_kernel_spmd

B, H, S, D = 4, 16, 2048, 64
N_CORES = 8
HPC = B * H // N_CORES     # heads per core
NPAIR = 8                  # k-tile pairs (16 k-tiles of 128)
QB = 512                   # q sub-block (one PSUM bank of scores per k-tile)
SB = 1024                  # q super-block (one stationary load per k-tile)
NSB = S // SB
DT = mybir.dt

# Schraudolph-in-bf16-bit-space constants: exp(score/8) ~= bits of
# int16(score * EXP_A + EXP_B). EXP_B calibrated for round-to-nearest.
EXP_A = 128.0 * 1.4426950408889634 / 8.0
EXP_B = 16248.5
# (pair, sub-block) slots whose exp runs on DVE instead of ACT (7 of 16).
DVE_SLOTS = ((1, 1), (2, 0), (3, 1), (4, 0), (5, 1), (6, 0), (7, 1))

# --- tuning knobs (env-overridable for A/B) ---
PV_ROW_TILED = os.environ.get("PV_ROW_TILED", "0") == "1"
WARMUP_MMS = int(os.environ.get("WARMUP_MMS", "8"))
ST_BUFS = int(os.environ.get("ST_BUFS", "2" if PV_ROW_TILED else "3"))
OT_BUFS = int(os.environ.get("OT_BUFS", "2"))
# interleave cadence: one PV chunk (8 MMs) per PV_EVERY QK pairs. Row-tiled
# PV has 64 MMs per super-block (vs 32 classic) so it interleaves per-pair.
PV_EVERY = 1 if PV_ROW_TILED else 2
# QK emission order: alternate row groups (T0,T8,T0,T8) vs baseline
# (T0,T0,T8,T8).
QK_ALT = os.environ.get("QK_ALT", "1") == "1"
# First-head DMA split across queues.
DMA_V2 = os.environ.get("DMA_V2", "1") == "1"

_BUILT = {}


class _Bacc(bacc.Bacc):
    """Bacc with the move-matmul-waits-to-ldweights pass disabled: keeping
    waits on the matmul (not its LDWEIGHTS) lets the PE queue pull weight
    loads ahead of in-flight matmuls, hiding most of the LDW cost, and
    avoids walrus folding LDW waits into the matmul's sync-wait budget."""

    def move_matmul_waits_to_ldweights(self):
        pass


def _head(nc, pools, scale, qt_d, kt_d, vp_d, h):
    """Emit head h's loads + per-super-block QK/exp. Yields None at each
    2-QK-group boundary (interleave point for the previous super-block's
    PV) and (vp, ets, q0) when a super-block's score tiles are emitted."""
    (stage, epool, spool, ps_st, ps_ot) = pools
    # qt rows 0:64 / 64:128 both hold Q^T. kt packs k-tile pairs: rows
    # 0:64 = K^T of tile 2j, rows 64:128 = K^T of tile 2j+1. vp is
    # partition-major: vp[p, t, e] = V'[t*128 + p, e].
    qt = stage.tile([128, S], DT.bfloat16, tag="qt")
    kt = stage.tile([128, NPAIR * 128], DT.bfloat16, tag="kt")
    vp = stage.tile([128, 2 * NPAIR, 128], DT.bfloat16, tag="vp")
    if h == 0 and DMA_V2:
        # first QK group needs only kt pair 0 (32KB) + qt[0:512]; land
        # those first (on two queues in parallel) so the PE starts early.
        nc.sync.dma_start(out=kt[:, 0:128], in_=kt_d[h][:, 0:128])
        nc.gpsimd.dma_start(out=qt[:, 0:QB], in_=qt_d[h][:, 0:QB])
        nc.sync.dma_start(out=kt[:, 128:], in_=kt_d[h][:, 128:])
        nc.sync.dma_start(out=qt[:, QB:SB], in_=qt_d[h][:, QB:SB])
        nc.gpsimd.dma_start(out=qt[:, SB:], in_=qt_d[h][:, SB:])
    elif h == 0:
        nc.sync.dma_start(out=kt[:, 0:128], in_=kt_d[h][:, 0:128])
        nc.sync.dma_start(out=qt[:, 0:QB], in_=qt_d[h][:, 0:QB])
        nc.sync.dma_start(out=kt[:, 128:], in_=kt_d[h][:, 128:])
        nc.gpsimd.dma_start(out=qt[:, QB:SB], in_=qt_d[h][:, QB:SB])
        nc.gpsimd.dma_start(out=qt[:, SB:], in_=qt_d[h][:, SB:])
    else:
        nc.gpsimd.dma_start(out=kt, in_=kt_d[h])
        for j in range(2):
            half = slice(j * (S // 2), (j + 1) * (S // 2))
            nc.gpsimd.dma_start(out=qt[:, half], in_=qt_d[h][:, half])
    nc.gpsimd.dma_start(out=vp,
                        in_=vp_d[h].rearrange("p (t e) -> p t e", e=128))

    for c in range(NSB):
        q0 = c * SB
        ets = []
        for j in range(NPAIR):
            if j % PV_EVERY == 0:
                yield None
            st0 = ps_st.tile([128, 2 * QB], DT.float32, tag="st")
            st1 = ps_st.tile([128, 2 * QB], DT.float32, tag="st")
            # T0: k-tile 2j via rows 0:64; T8: k-tile 2j+1 via rows
            # 64:128, concurrent. Sub-blocks fill each st tile's 2 banks.
            # With QK_ALT, row groups alternate (T0,T8,T0,T8) so every
            # LDWEIGHTS (bass re-emits one per matmul, even for repeated
            # weights) overlaps the other group's in-flight matmul.
            def _qk(s, st, lo):
                rows = slice(0, 64) if lo else slice(64, 128)
                nc.tensor.matmul(
                    st[:, 0:QB] if lo else st[:, QB : 2 * QB],
                    lhsT=kt[rows, j * 128 : (j + 1) * 128],
                    rhs=qt[rows, q0 + s * QB : q0 + (s + 1) * QB],
                    start=True, stop=True,
                )
            if QK_ALT:
                for s, st in enumerate((st0, st1)):
                    _qk(s, st, True)
                    _qk(s, st, False)
            else:
                for s, st in enumerate((st0, st1)):
                    _qk(s, st, True)
                for s, st in enumerate((st0, st1)):
                    _qk(s, st, False)
            for s, st in enumerate((st0, st1)):
                et = epool.tile([128, 2 * QB], DT.bfloat16, tag=f"et{j}_{s}")
                if (j, s) in DVE_SLOTS:
                    nc.vector.tensor_scalar(
                        et.bitcast(DT.int16), st, EXP_A, EXP_B,
                        mybir.AluOpType.mult, mybir.AluOpType.add,
                    )
                else:
                    nc.scalar.activation(
                        out=et, in_=st,
                        func=mybir.ActivationFunctionType.Exp, scale=scale,
                    )
                ets.append(et)
        yield (vp, list(ets), q0)


def _pv_gen_row(nc, pools, vp, ets, o_ap, q0):
    """Row-tiled PV for one super-block (64x128 mode, same as QK => no PE
    tiling-mode switches). V' tile t is split into k-rows 0:64 (T0) and
    64:128 (T8); both halves stream their et half concurrently. T0/T8
    accumulate into separate 2-bank PSUM tiles (cols 0:512 = sub-block 0,
    512:1024 = sub-block 1). Merged at the end by an ACT Copy (lo -> SBUF)
    plus a DVE add (SBUF + hi -> SBUF): a DVE op may read at most one PSUM
    operand, so the two partial banks cannot be added in one instruction.
    Yields every 8 matmuls."""
    (stage, epool, spool, ps_st, ps_ot) = pools
    ot_lo = ps_ot.tile([128, 2 * QB], DT.float32, tag="ot")
    ot_hi = ps_ot.tile([128, 2 * QB], DT.float32, tag="ot")
    k = 0
    for j in range(NPAIR):
        for half in range(2):
            t = 2 * j + half
            for s in range(2):
                et = ets[2 * j + s]
                for g, (rows, ot) in enumerate(
                        ((slice(0, 64), ot_lo), (slice(64, 128), ot_hi))):
                    nc.tensor.matmul(
                        ot[:, s * QB : (s + 1) * QB],
                        lhsT=vp[rows, t, :],
                        rhs=et[rows, half * QB : (half + 1) * QB],
                        start=(t == 0), stop=(t == 2 * NPAIR - 1),
                    )
                    k += 1
                    if k % 8 == 0:
                        yield
    # unnormalized out^T: rows 0:64 numerator, row 64 denominator.
    otmp = spool.tile([D + 1, 2 * QB], DT.float32, tag="otmp")
    nc.scalar.activation(out=otmp, in_=ot_lo[0 : D + 1, :],
                         func=mybir.ActivationFunctionType.Copy)
    ots = spool.tile([D + 1, 2 * QB], DT.float32, tag="ots")
    nc.vector.tensor_tensor(out=ots, in0=otmp, in1=ot_hi[0 : D + 1, :],
                            op=mybir.AluOpType.add)
    nc.sync.dma_start(out=o_ap[:, q0 : q0 + SB], in_=ots)


def _pv_gen_classic(nc, pools, vp, ets, o_ap, q0):
    """Baseline PV (128x128 mode), yielding every 8 matmuls."""
    (stage, epool, spool, ps_st, ps_ot) = pools
    k = 0
    for s in range(2):
        ot = ps_ot.tile([128, QB], DT.float32, tag="ot")
        for j in range(NPAIR):
            for half in range(2):
                t = 2 * j + half
                nc.tensor.matmul(
                    ot,
                    lhsT=vp[:, t, :],
                    rhs=ets[2 * j + s][:, half * QB : (half + 1) * QB],
                    start=(t == 0), stop=(t == 2 * NPAIR - 1),
                )
                k += 1
                if k % 8 == 0:
                    yield
        ots = spool.tile([D + 1, QB], DT.float32, tag="ots")
        nc.vector.tensor_copy(out=ots, in_=ot[0 : D + 1, :])
        nc.sync.dma_start(out=o_ap[:, q0 + s * QB : q0 + (s + 1) * QB],
                          in_=ots)


def build_graph(scale: float, heads: int = HPC):
    nc = _Bacc("TRN2", target_bir_lowering=False, debug=False,
               num_devices=N_CORES)
    qt_d = nc.dram_tensor("QT", [heads, 128, S], DT.bfloat16,
                          kind="ExternalInput").ap()
    kt_d = nc.dram_tensor("KT", [heads, 128, NPAIR * 128], DT.bfloat16,
                          kind="ExternalInput").ap()
    vp_d = nc.dram_tensor("VP", [heads, 128, 2 * NPAIR * 128], DT.bfloat16,
                          kind="ExternalInput").ap()
    o_d = nc.dram_tensor("out", [heads, D + 1, S], DT.float32,
                         kind="ExternalOutput").ap()

    pv_gen = _pv_gen_row if PV_ROW_TILED else _pv_gen_classic

    with tile.TileContext(nc) as tc, ExitStack() as ctx:
        stage = ctx.enter_context(tc.tile_pool(name="stage", bufs=2))
        epool = ctx.enter_context(tc.tile_pool(name="epool", bufs=2))
        spool = ctx.enter_context(tc.tile_pool(name="spool", bufs=4))
        ps_st = ctx.enter_context(tc.tile_pool(name="ps_st", bufs=ST_BUFS,
                                               space="PSUM"))
        ps_ot = ctx.enter_context(tc.tile_pool(name="ps_ot", bufs=OT_BUFS,
                                               space="PSUM"))

        pools = (stage, epool, spool, ps_st, ps_ot)

        if WARMUP_MMS:
            # Warm the PE_HAM clock gate during the initial DMA wait: dummy
            # matmuls on a memset scratch tile into a recycled psum tile.
            wsrc = stage.tile([128, 128 + QB], DT.bfloat16, tag="warm")
            nc.vector.memset(wsrc, 0.0)
            wps = ps_st.tile([128, 2 * QB], DT.float32, tag="st")
            for _ in range(WARMUP_MMS):
                nc.tensor.matmul(wps[:, 0:QB], lhsT=wsrc[:, 0:128],
                                 rhs=wsrc[:, 128:], start=True, stop=True)

        prev = None
        for h in range(HPC):
            for item in _head(nc, pools, scale, qt_d, kt_d, vp_d, h):
                if item is None:
                    if prev is not None:
                        next(prev, None)   # 8 PV MMs of super-block g-1
                    continue
                if prev is not None:
                    for _ in prev:         # PV tail + merges + stores
                        pass
                prev = pv_gen(nc, pools, item[0], item[1], o_d[h], item[2])
        if prev is not None:
            for _ in prev:                 # drain the final super-block
                pass
    nc.compile()
    return nc


def _get_nc(scale: float):
    key = (round(float(scale), 9), PV_ROW_TILED, WARMUP_MMS, ST_BUFS, OT_BUFS)
    if key not in _BUILT:
        _BUILT[key] = build_graph(float(scale))
    return _BUILT[key]


def shard_inputs(Q, K, V):
    """Host-side prep: shard heads across cores; build qt (Q^T duplicated
    into both 64-row halves), kt (k-tile pairs packed for row tiling), vp
    (V plus a ones column, partition-major so every DMA is contiguous)."""
    bf16 = ml_dtypes.bfloat16
    qs = np.asarray(Q, dtype=np.float32).reshape(B * H, S, D)
    ks = np.asarray(K, dtype=np.float32).reshape(B * H, S, D)
    vs = np.asarray(V, dtype=np.float32).reshape(B * H, S, D)
    qtT = qs.transpose(0, 2, 1).astype(bf16)           # [BH, D, S]
    qt = np.concatenate([qtT, qtT], axis=1)            # [BH, 128, S]
    ktT = ks.transpose(0, 2, 1).astype(bf16)           # [BH, D, S]
    ktv = ktT.reshape(B * H, D, NPAIR, 2, 128)
    kt = np.empty((B * H, 128, NPAIR, 128), dtype=bf16)
    kt[:, :D] = ktv[:, :, :, 0, :]                     # rows 0:64 <- 2j
    kt[:, D:] = ktv[:, :, :, 1, :]                     # rows 64:128 <- 2j+1
    kt = kt.reshape(B * H, 128, NPAIR * 128)
    vpb = np.zeros((B * H, S, 128), dtype=bf16)
    vpb[:, :, :D] = vs.astype(bf16)
    vpb[:, :, D] = np.float32(1.0)
    # vp[bh, p, t*128 + e] = V'[bh, t*128 + p, e]
    vp = (vpb.reshape(B * H, 2 * NPAIR, 128, 128)
          .transpose(0, 2, 1, 3).reshape(B * H, 128, 2 * NPAIR * 128))
    in_maps = []
    for c in range(N_CORES):
        sl = slice(c * HPC, (c + 1) * HPC)
        in_maps.append({
            "QT": np.ascontiguousarray(qt[sl]),
            "KT": np.ascontiguousarray(kt[sl]),
            "VP": np.ascontiguousarray(vp[sl]),
        })
    return in_maps


def kernel(Q, K, V, d_k, **run_kwargs):
    scale = 1.0 / math.sqrt(float(d_k))
    nc = _get_nc(scale)
    in_maps = shard_inputs(Q, K, V)
    res = run_bass_kernel_spmd(nc, in_maps, core_ids=list(range(N_CORES)),
                               **run_kwargs)
    # device output is [heads, 65, S]: rows 0:64 = (sum_k p*V)^T, row 64 =
    # softmax denominator. Normalize + transpose on the host.
    outs = []
    for r in res.results:
        o = r["out"]
        outs.append((o[:, :D, :] / o[:, D : D + 1, :]).transpose(0, 2, 1))
    out = np.concatenate(outs, axis=0).reshape(B, H, S, D)
    out = np.ascontiguousarray(out, dtype=np.float32)
    kernel.last_results = res
    return out
